# revision 16
# baseline (speedup 1.0000x reference)
"""CompGCNConv kernel for 8 Trainium2 NeuronCores (Bass/Tile).

Math (see reference):
  out = BN(ReLU-pre) of [ ent @ W_S.T + A_fwd @ W_O.T + A_inv @ W_I.T ] / deg
  A_fwd[n] = sum_{e: dst[e]=n} (ent[src[e]] - rel[type[e]])
  A_inv[n] = sum_{e: src[e]=n} (ent[dst[e]] - rel[type[e]])
  new_rel = rel @ W_rel.T

Key restructuring: matmuls are linear, so aggregate first, multiply after.
The relation part of each aggregate is H @ rel_emb where H is a per-node
relation-type histogram (host bincount).  The entity part is a gather +
segmented-sum, done on device as: dma_gather of entity rows (bf16) and a
one-hot "selection" matmul per 128-edge tile accumulating A^T per 128-row
output block in PSUM.  Each core owns 1/8 of the output rows; edges are
binned by owner octant/block on the host (counting sort).  Degree division
is folded into the PSUM->SBUF drain.

BatchNorm needs global per-feature stats; device collectives are broken on
this stack, so the kernel is split: program A produces pre-BN activations
(feature-major, on device) + per-core stat partials (tiny), the host sums
8x[128,2] and computes scale/bias columns, program B applies
scale/bias+ReLU, transposes, and writes the output rows.
"""

import sys

sys.path.insert(0, "/opt/trn_rl_repo")

from contextlib import ExitStack
from dataclasses import dataclass

import ml_dtypes
import numpy as np

import concourse.bacc as bacc
import concourse.bass as bass
import concourse.mybir as mybir
import concourse.tile as tile
from concourse import library_config
from concourse.bass_utils import run_bass_kernel_spmd

F32 = mybir.dt.float32
BF16 = mybir.dt.bfloat16
I16 = mybir.dt.int16
I32 = mybir.dt.int32
BF = ml_dtypes.bfloat16

PB = 128  # partition/block size


@dataclass
class Cfg:
    n_ent: int  # true number of entities
    n_rel: int
    ncores: int
    nblk: int  # 128-row blocks per core
    ta: int  # tiles per block: fwd
    tc: int  # tiles per block: inv
    bn_eps: float = 1e-5

    @property
    def bn(self):  # padded rows per core
        return self.nblk * PB

    @property
    def padn(self):
        return self.ncores * self.bn


def _wrap_idx(a: np.ndarray) -> np.ndarray:
    """int16 [L] -> [128, L/16] layout for dma_gather (16-wrap, 8x replicated)."""
    assert a.size % 16 == 0
    w = a.reshape(-1, 16).T  # [16, L/16]
    return np.ascontiguousarray(np.tile(w, (8, 1)))


def _tile_major(a: np.ndarray) -> np.ndarray:
    """[ntiles*128] -> [128, ntiles] (column t = tile t, row p = slot p)."""
    return np.ascontiguousarray(a.reshape(-1, PB).T)


def _build_dir(ncores, nblk, bn, owner, gather, relr_all):
    """Bin edges by (owner octant, block); pad each bin to x128 tiles.

    Returns (ta, idx[nc,*] int32 gather rows, dr[nc,*] fp32 within-block row
    ids; pads gather row 0 with dr=-5 which matches no one-hot row)."""
    e = owner.size
    oct_ = owner // bn
    blk = (owner % bn) // PB
    key = oct_ * nblk + blk
    order = np.argsort(key, kind="stable")
    cnt = np.bincount(key, minlength=ncores * nblk)
    ta = max(1, int(np.ceil(cnt.max() / PB)))
    starts = np.zeros(cnt.size + 1, np.int64)
    starts[1:] = np.cumsum(cnt)
    pos = np.arange(e) - starts[key[order]]
    sg, sr = gather[order], relr_all[order]
    so, sb_ = oct_[order], blk[order]
    idx = np.zeros((ncores, nblk * ta * PB), np.int32)
    dr = np.full((ncores, nblk * ta * PB), -5.0, np.float32)
    slot = sb_ * ta * PB + pos
    idx[so, slot] = sg.astype(np.int32)
    dr[so, slot] = sr.astype(np.float32)
    return ta, idx, dr


def prepare(entity_emb, relation_emb, edge_index, edge_type,
            W_O, W_I, W_S, W_rel, gamma, beta,
            ncores=8, nblk=49):
    """Host preprocessing -> (cfg, in_maps_a, gamma, beta)."""
    n_ent, d = entity_emb.shape
    assert d == PB
    n_rel = relation_emb.shape[0]
    src = np.asarray(edge_index[0], np.int64)
    dst = np.asarray(edge_index[1], np.int64)
    typ = np.asarray(edge_type, np.int64)

    bn = nblk * PB
    padn = ncores * bn
    assert padn >= n_ent

    deg = np.bincount(dst, minlength=padn) + np.bincount(src, minlength=padn)
    deg = np.maximum(deg, 1).astype(np.float64)
    recip = (1.0 / deg).astype(np.float32)

    ta, il_f, dl_f = _build_dir(ncores, nblk, bn, dst, src, (dst % bn) % PB)
    tc_, il_i, dl_i = _build_dir(ncores, nblk, bn, src, dst, (src % bn) % PB)
    cfg = Cfg(n_ent=n_ent, n_rel=n_rel, ncores=ncores, nblk=nblk,
              ta=ta, tc=tc_)

    # per-node relation-type histograms, pre-scaled by 1/deg, transposed
    hf = np.bincount(dst * n_rel + typ, minlength=padn * n_rel).reshape(padn, n_rel)
    hi_ = np.bincount(src * n_rel + typ, minlength=padn * n_rel).reshape(padn, n_rel)
    hf = (hf * recip[:, None]).astype(np.float32)
    hi_ = (hi_ * recip[:, None]).astype(np.float32)

    ent_pad = np.zeros((padn, d), np.float32)
    ent_pad[:n_ent] = np.asarray(entity_emb, np.float32)
    ent_bf = ent_pad.astype(BF)

    rel = np.asarray(relation_emb, np.float32)
    W_O = np.asarray(W_O, np.float32)
    W_I = np.asarray(W_I, np.float32)
    W_S = np.asarray(W_S, np.float32)
    W_rel = np.asarray(W_rel, np.float32)
    rel_o_neg = -(rel @ W_O.T)  # [R,128]
    rel_i_neg = -(rel @ W_I.T)
    r1 = min(PB, n_rel)

    iota = np.tile(np.arange(PB, dtype=np.float32), (PB, 1))

    def rel_chunks(m):  # [R,128] -> ([r1,128], [R-r1,128] or None)
        return (np.ascontiguousarray(m[:r1]).astype(BF),
                np.ascontiguousarray(m[r1:]).astype(BF) if n_rel > r1 else None)

    ro1, ro2 = rel_chunks(rel_o_neg)
    ri1, ri2 = rel_chunks(rel_i_neg)

    in_maps = []
    for c in range(ncores):
        rows = slice(c * bn, (c + 1) * bn)
        entoct_t = ((ent_pad[rows] * recip[rows, None]).T).astype(BF)
        m = {
            "ent_bf": ent_bf,
            "idx_f": _tile_major(il_f[c]), "idx_i": _tile_major(il_i[c]),
            "dr_f": _tile_major(dl_f[c]), "dr_i": _tile_major(dl_i[c]),
            "hft1": np.ascontiguousarray(hf[rows, :r1].T).astype(BF),
            "hit1": np.ascontiguousarray(hi_[rows, :r1].T).astype(BF),
            "entoct_t": np.ascontiguousarray(entoct_t),
            "recip_rep": np.ascontiguousarray(
                np.broadcast_to(recip[rows], (PB, bn))).astype(BF),
            "wot": W_O.T.astype(BF).copy(), "wit": W_I.T.astype(BF).copy(),
            "wst": W_S.T.astype(BF).copy(),
            "ro1": ro1, "ri1": ri1,
            "rel_t": np.ascontiguousarray(rel.T).astype(np.float32),  # [128,R]
            "wrel_t": W_rel.T.astype(np.float32).copy(),
            "iota": iota,
        }
        if ro2 is not None:
            m["ro2"] = ro2
            m["ri2"] = ri2
            m["hft2"] = np.ascontiguousarray(hf[rows, r1:].T).astype(BF)
            m["hit2"] = np.ascontiguousarray(hi_[rows, r1:].T).astype(BF)
        in_maps.append(m)
    return cfg, in_maps, np.asarray(gamma, np.float64), np.asarray(beta, np.float64)


def build_program_a(cfg: Cfg):
    """Aggregation + term combination -> pre-BN out_t (feature-major) +
    per-core BN stat partials + new_rel."""
    nc = bacc.Bacc("TRN2", target_bir_lowering=False, debug=False,
                   num_devices=cfg.ncores)
    nblk, bn, padn, r = cfg.nblk, cfg.bn, cfg.padn, cfg.n_rel
    r1 = min(PB, r)
    r2 = r - r1

    ent_bf = nc.dram_tensor("ent_bf", [padn, PB], BF16, kind="ExternalInput")
    di = {}
    for nm, t in (("f", cfg.ta), ("i", cfg.tc)):
        di["idx_" + nm] = nc.dram_tensor(
            "idx_" + nm, [PB, nblk * t], I32, kind="ExternalInput")
        di["dr_" + nm] = nc.dram_tensor(
            "dr_" + nm, [PB, nblk * t], F32, kind="ExternalInput")
    hft1 = nc.dram_tensor("hft1", [r1, bn], BF16, kind="ExternalInput")
    hit1 = nc.dram_tensor("hit1", [r1, bn], BF16, kind="ExternalInput")
    if r2:
        hft2 = nc.dram_tensor("hft2", [r2, bn], BF16, kind="ExternalInput")
        hit2 = nc.dram_tensor("hit2", [r2, bn], BF16, kind="ExternalInput")
    entoct_t = nc.dram_tensor("entoct_t", [PB, bn], BF16, kind="ExternalInput")
    recip_rep = nc.dram_tensor("recip_rep", [PB, bn], BF16, kind="ExternalInput")
    wot = nc.dram_tensor("wot", [PB, PB], BF16, kind="ExternalInput")
    wit = nc.dram_tensor("wit", [PB, PB], BF16, kind="ExternalInput")
    wst = nc.dram_tensor("wst", [PB, PB], BF16, kind="ExternalInput")
    ro1 = nc.dram_tensor("ro1", [r1, PB], BF16, kind="ExternalInput")
    ri1 = nc.dram_tensor("ri1", [r1, PB], BF16, kind="ExternalInput")
    if r2:
        ro2 = nc.dram_tensor("ro2", [r2, PB], BF16, kind="ExternalInput")
        ri2 = nc.dram_tensor("ri2", [r2, PB], BF16, kind="ExternalInput")
    rel_t = nc.dram_tensor("rel_t", [PB, r], F32, kind="ExternalInput")
    wrel_t = nc.dram_tensor("wrel_t", [PB, PB], F32, kind="ExternalInput")
    iota_d = nc.dram_tensor("iota", [PB, PB], F32, kind="ExternalInput")

    out_t_ext = nc.dram_tensor("out_t", [PB, bn], BF16, kind="ExternalOutput")
    st_ext = nc.dram_tensor("st", [PB, 2], F32, kind="ExternalOutput")
    newrel = nc.dram_tensor("newrel", [r, PB], F32, kind="ExternalOutput")

    with tile.TileContext(nc) as tc, ExitStack() as ctx:
        consts = ctx.enter_context(tc.tile_pool(name="consts", bufs=1))
        gpool = ctx.enter_context(tc.tile_pool(name="gpool", bufs=24))
        spool = ctx.enter_context(tc.tile_pool(name="spool", bufs=6))
        stg = ctx.enter_context(tc.tile_pool(name="stg", bufs=3))
        psa = ctx.enter_context(tc.tile_pool(name="psa", bufs=2, space="PSUM"))
        psb = ctx.enter_context(tc.tile_pool(name="psb", bufs=3, space="PSUM"))

        def load(dram, shape, dtype):
            t = consts.tile(shape, dtype, tag=dram.name)
            nc.sync.dma_start(out=t[:], in_=dram[:])
            return t

        sb = {}
        for nm, t in (("f", cfg.ta), ("i", cfg.tc)):
            sb["idx_" + nm] = load(di["idx_" + nm], [PB, nblk * t], I32)
            sb["dr_" + nm] = load(di["dr_" + nm], [PB, nblk * t], F32)
        recip_s = load(recip_rep, [PB, bn], BF16)
        wot_s = load(wot, [PB, PB], BF16)
        wit_s = load(wit, [PB, PB], BF16)
        wst_s = load(wst, [PB, PB], BF16)
        ro1_s = load(ro1, [r1, PB], BF16)
        ri1_s = load(ri1, [r1, PB], BF16)
        if r2:
            ro2_s = load(ro2, [r2, PB], BF16)
            ri2_s = load(ri2, [r2, PB], BF16)
        relt_s = load(rel_t, [PB, r], F32)
        wrelt_s = load(wrel_t, [PB, PB], F32)
        iota_s = load(iota_d, [PB, PB], F32)

        acc_f = consts.tile([PB, bn], BF16, tag="acc_f")
        acc_i = consts.tile([PB, bn], BF16, tag="acc_i")
        out_t = consts.tile([PB, bn], BF16, tag="out_t")
        s1 = consts.tile([PB, nblk], F32, tag="s1")
        s2 = consts.tile([PB, nblk], F32, tag="s2")

        # ---- phase 1/2: per-tile indirect gather + one-hot scatter matmul ----
        for dirn in ("f", "i"):
            tpb = cfg.ta if dirn == "f" else cfg.tc
            acc = acc_f if dirn == "f" else acc_i
            idx_sb, dr_sb = sb["idx_" + dirn], sb["dr_" + dirn]
            for b in range(nblk):
                pa = psa.tile([PB, PB], F32, tag="pa")
                for t in range(tpb):
                    gt = b * tpb + t
                    g = gpool.tile([PB, PB], BF16, tag="g")
                    nc.gpsimd.indirect_dma_start(
                        out=g[:], out_offset=None, in_=ent_bf[:, :],
                        in_offset=bass.IndirectOffsetOnAxis(
                            ap=idx_sb[:, gt:gt + 1], axis=0))
                    s = spool.tile([PB, PB], BF16, tag="s")
                    nc.vector.tensor_tensor(
                        out=s[:], in0=dr_sb[:, gt:gt + 1].to_broadcast([PB, PB]),
                        in1=iota_s[:], op=mybir.AluOpType.is_equal)
                    nc.tensor.matmul(
                        out=pa[:], lhsT=g[:], rhs=s[:],
                        start=(t == 0), stop=(t == tpb - 1))
                nc.vector.tensor_tensor(
                    out=acc[:, b * PB:(b + 1) * PB], in0=pa[:],
                    in1=recip_s[:, b * PB:(b + 1) * PB],
                    op=mybir.AluOpType.mult)

        # ---- phase 3: combine terms per block; BN partial stats ----
        hpool = ctx.enter_context(tc.tile_pool(name="hpool", bufs=3))
        for b in range(nblk):
            bsl = slice(b * PB, (b + 1) * PB)
            hf1b = hpool.tile([r1, PB], BF16, tag="hf1b")
            nc.sync.dma_start(out=hf1b[:], in_=hft1[:, bsl])
            hi1b = hpool.tile([r1, PB], BF16, tag="hi1b")
            nc.sync.dma_start(out=hi1b[:], in_=hit1[:, bsl])
            if r2:
                hf2b = hpool.tile([r2, PB], BF16, tag="hf2b")
                nc.sync.dma_start(out=hf2b[:], in_=hft2[:, bsl])
                hi2b = hpool.tile([r2, PB], BF16, tag="hi2b")
                nc.sync.dma_start(out=hi2b[:], in_=hit2[:, bsl])
            eob = hpool.tile([PB, PB], BF16, tag="eob")
            nc.sync.dma_start(out=eob[:], in_=entoct_t[:, bsl])
            po = psb.tile([PB, PB], F32, tag="po")
            nc.tensor.matmul(out=po[:], lhsT=wot_s[:], rhs=acc_f[:, bsl],
                             start=True, stop=False)
            nc.tensor.matmul(out=po[:], lhsT=wit_s[:], rhs=acc_i[:, bsl],
                             start=False, stop=False)
            nc.tensor.matmul(out=po[:], lhsT=wst_s[:], rhs=eob[:],
                             start=False, stop=False)
            nc.tensor.matmul(out=po[:], lhsT=ro1_s[:], rhs=hf1b[:],
                             start=False, stop=False)
            nc.tensor.matmul(out=po[:], lhsT=ri1_s[:], rhs=hi1b[:],
                             start=False, stop=(r2 == 0))
            if r2:
                nc.tensor.matmul(out=po[:], lhsT=ro2_s[:], rhs=hf2b[:],
                                 start=False, stop=False)
                nc.tensor.matmul(out=po[:], lhsT=ri2_s[:], rhs=hi2b[:],
                                 start=False, stop=True)
            nc.scalar.activation(
                out=out_t[:, bsl], in_=po[:],
                func=mybir.ActivationFunctionType.Copy,
                accum_out=s1[:, b:b + 1])
            sq = stg.tile([PB, PB], BF16, tag="sq")
            nc.scalar.activation(
                out=sq[:], in_=po[:],
                func=mybir.ActivationFunctionType.Square,
                accum_out=s2[:, b:b + 1])

        # ---- stats partials out ----
        st = consts.tile([PB, 2], F32, tag="st")
        nc.vector.tensor_reduce(out=st[:, 0:1], in_=s1[:], axis=mybir.AxisListType.X,
                                op=mybir.AluOpType.add)
        nc.vector.tensor_reduce(out=st[:, 1:2], in_=s2[:], axis=mybir.AxisListType.X,
                                op=mybir.AluOpType.add)
        nc.sync.dma_start(out=st_ext[:, :], in_=st[:])
        nc.sync.dma_start(out=out_t_ext[:, :], in_=out_t[:])

        # ---- new_rel = rel @ W_rel.T (fp32) ----
        pr = psb.tile([r1, PB], F32, tag="po")
        nc.tensor.matmul(out=pr[:], lhsT=relt_s[:, :r1], rhs=wrelt_s[:],
                         start=True, stop=True)
        nr1 = stg.tile([r1, PB], F32, tag="nr1")
        nc.vector.tensor_copy(out=nr1[:], in_=pr[:])
        nc.sync.dma_start(out=newrel[:r1, :], in_=nr1[:])
        if r2:
            pr2 = psb.tile([r2, PB], F32, tag="po")
            nc.tensor.matmul(out=pr2[:], lhsT=relt_s[:, r1:], rhs=wrelt_s[:],
                             start=True, stop=True)
            nr2 = stg.tile([r2, PB], F32, tag="nr2")
            nc.vector.tensor_copy(out=nr2[:], in_=pr2[:])
            nc.sync.dma_start(out=newrel[r1:, :], in_=nr2[:])

    nc.compile()
    return nc


def build_program_b(cfg: Cfg):
    """BN scale/bias + ReLU + transpose + row-major write."""
    nc = bacc.Bacc("TRN2", target_bir_lowering=False, debug=False,
                   num_devices=cfg.ncores)
    nblk, bn = cfg.nblk, cfg.bn
    out_t_in = nc.dram_tensor("out_t", [PB, bn], BF16, kind="ExternalInput")
    scale_in = nc.dram_tensor("scale_c", [PB, 1], F32, kind="ExternalInput")
    bias_in = nc.dram_tensor("bias_c", [PB, 1], F32, kind="ExternalInput")
    ident_d = nc.dram_tensor("ident", [PB, PB], F32, kind="ExternalInput")
    out_oct = nc.dram_tensor("out_oct", [bn, PB], F32, kind="ExternalOutput")

    with tile.TileContext(nc) as tc, ExitStack() as ctx:
        consts = ctx.enter_context(tc.tile_pool(name="consts", bufs=1))
        stg = ctx.enter_context(tc.tile_pool(name="stg", bufs=4))
        ps = ctx.enter_context(tc.tile_pool(name="ps", bufs=3, space="PSUM"))
        xt = consts.tile([PB, bn], BF16, tag="xt")
        nc.sync.dma_start(out=xt[:], in_=out_t_in[:])
        scale_s = consts.tile([PB, 1], F32, tag="scale_s")
        nc.sync.dma_start(out=scale_s[:], in_=scale_in[:])
        bias_s = consts.tile([PB, 1], F32, tag="bias_s")
        nc.sync.dma_start(out=bias_s[:], in_=bias_in[:])
        ident_s = consts.tile([PB, PB], F32, tag="ident_s")
        nc.sync.dma_start(out=ident_s[:], in_=ident_d[:])
        for b in range(nblk):
            bsl = slice(b * PB, (b + 1) * PB)
            r1b = stg.tile([PB, PB], F32, tag="r1b")
            nc.scalar.activation(out=r1b[:], in_=xt[:, bsl],
                                 func=mybir.ActivationFunctionType.Relu,
                                 bias=bias_s[:], scale=scale_s[:])
            pt = ps.tile([PB, PB], F32, tag="pt")
            nc.tensor.transpose(pt[:], r1b[:], ident_s[:])
            o1 = stg.tile([PB, PB], F32, tag="o1")
            nc.vector.tensor_copy(out=o1[:], in_=pt[:])
            nc.sync.dma_start(out=out_oct[b * PB:(b + 1) * PB, :], in_=o1[:])
    nc.compile()
    return nc


def host_stats(st_sum, gamma, beta, n_ent, eps):
    """st_sum [128,2] (sum, sumsq over all rows) -> scale/bias columns."""
    mean = st_sum[:, 0] / n_ent
    var = st_sum[:, 1] / n_ent - mean * mean
    scale = np.asarray(gamma).reshape(-1) / np.sqrt(var + eps)
    bias = np.asarray(beta).reshape(-1) - mean * scale
    return (scale.astype(np.float32).reshape(PB, 1).copy(),
            bias.astype(np.float32).reshape(PB, 1).copy())


_CACHE = {}


def get_programs(cfg: Cfg):
    key = (cfg.n_ent, cfg.n_rel, cfg.ncores, cfg.nblk, cfg.ta, cfg.tc)
    if key not in _CACHE:
        _CACHE[key] = (build_program_a(cfg), build_program_b(cfg))
    return _CACHE[key]


def run(inputs: dict):
    cfg, in_maps, gamma, beta = prepare(**inputs)
    nc_a, nc_b = get_programs(cfg)
    cores = list(range(cfg.ncores))
    res_a = run_bass_kernel_spmd(nc_a, in_maps, cores)
    outs_a = res_a.results
    st_sum = np.sum([outs_a[c]["st"].astype(np.float64) for c in cores], axis=0)
    scale_c, bias_c = host_stats(st_sum, gamma, beta, cfg.n_ent, cfg.bn_eps)
    ident = np.eye(PB, dtype=np.float32)
    in_maps_b = [{"out_t": outs_a[c]["out_t"], "scale_c": scale_c,
                  "bias_c": bias_c, "ident": ident} for c in cores]
    res_b = run_bass_kernel_spmd(nc_b, in_maps_b, cores)
    outs_b = res_b.results
    out = np.concatenate([outs_b[c]["out_oct"] for c in cores], axis=0)
    out = np.ascontiguousarray(out[:cfg.n_ent]).astype(np.float32)
    new_rel = outs_a[0]["newrel"].astype(np.float32)
    return out, new_rel


def kernel(**inputs):
    return run(inputs)
